# revision 1
# baseline (speedup 1.0000x reference)
"""Trainium2 Bass kernel for sheaf Dirichlet energy (ConsistencyBasedLaplacianBuilder).

loss = sum_e || maps[rev(e)] @ x[tgt(e)] - maps[e] @ x[src(e)] ||_F^2

Strategy (edge parallelism across 8 NeuronCores):
  The reference edge set is symmetric: edge e < H (=E/2) has its reverse at
  e + H, so the loss equals 2 * sum_{e<H} ||maps[e+H] x[dst] - maps[e] x[src]||^2.
  Each core takes a contiguous slice of the H half-edges, keeps a full replica
  of x in HBM, gathers x rows via indirect DMA (128 edges per tile, one edge
  per partition), and contracts on the vector engine with three wide ops per
  tile:
    prod[e, i, jj, f] = mcat[e, i, jj] * xcat[e, jj, f]      (f broadcast)
    diff[e, (i f)]    = sum_jj prod[e, i, jj, f]             (strided reduce)
    acc[e, tile]      = sum(diff * diff)                     (fused square+sum)
  where xcat = [x_dst | x_src] (jj in 0..7) and mcat interleaves maps_hi with
  negated maps_lo so the jj-sum forms the difference directly.
  Per-core partial sums are added on the host.
"""

import sys
import types

import numpy as np

sys.path.insert(0, "/opt/trn_rl_repo")

N = 50000
D = 4
F = 16
DF = D * F            # 64 floats per node row
E = 1600000
H = E // 2            # 800000 undirected pairs
NCORES = 8
EPC = H // NCORES     # 100000 half-edges per core

GROUP = 8             # tiles gathered per dma_gather pair
PAIR = 2 * GROUP      # tiles per loop iteration (double-buffered)
NT_USED = 800         # tiles per core (800*128 = 102400 >= 100000)
NT_ALLOC = 832        # padded columns (overhang gather reads into padding)
EPC_PAD = NT_USED * 128
# x is gathered with int16 indices (dma_gather), so it is split into two
# tables of XSPLIT+1 rows; row XSPLIT of each table is zero (out-of-range
# slot), and the two gathered halves are added.
XSPLIT = 25000


def _inject_axon_hooks():
    """The container's antenv lacks axon_hooks; provide it so NTFF tracing
    (used by test.py, harmless otherwise) can register."""
    if "antenv.axon_hooks" in sys.modules:
        return
    mod = types.ModuleType("antenv.axon_hooks")
    mod._hook = None

    def set_axon_ntff_profile_hook(h):
        mod._hook = h

    def get_axon_ntff_profile_hook():
        return mod._hook

    mod.set_axon_ntff_profile_hook = set_axon_ntff_profile_hook
    mod.get_axon_ntff_profile_hook = get_axon_ntff_profile_hook
    sys.modules["antenv.axon_hooks"] = mod


def _build_program(nt_used=NT_USED, nt_alloc=NT_ALLOC, n_nodes=N, ncores=NCORES):
    import concourse.bacc as bacc
    import concourse.bass as bass
    import concourse.tile as tile
    from concourse import mybir

    AP = bass.AP
    f32 = mybir.dt.float32
    i32 = mybir.dt.int32
    Op = mybir.AluOpType
    ds = bass.ds

    ngroups = nt_used // GROUP
    assert ngroups % 2 == 0
    niters = ngroups // 2

    i16 = mybir.dt.int16

    nc = bacc.Bacc("TRN2", target_bir_lowering=False, debug=False,
                   num_devices=ncores)

    xlo_d = nc.dram_tensor("xlo", [XSPLIT + 1, DF], f32, kind="ExternalInput")
    xhi_d = nc.dram_tensor("xhi", [XSPLIT + 1, DF], f32, kind="ExternalInput")
    mcat_d = nc.dram_tensor("mcat", [128, nt_alloc * 32], f32,
                            kind="ExternalInput")
    # int16 gather index streams in dma_gather wrapped layout: linear index
    # i = s*16 + p over [16, S], replicated 8x down the 128 partitions.
    # Linear order: block 2t = dst rows of tile t, block 2t+1 = src rows.
    glo_d = nc.dram_tensor("glo", [128, nt_alloc * 16], i16,
                           kind="ExternalInput")
    ghi_d = nc.dram_tensor("ghi", [128, nt_alloc * 16], i16,
                           kind="ExternalInput")
    loss_d = nc.dram_tensor("loss", [1, 1], f32, kind="ExternalOutput")

    with tile.TileContext(nc) as tc, \
         tc.tile_pool(name="persist", bufs=1) as pp, \
         tc.tile_pool(name="gather", bufs=1) as gp, \
         tc.tile_pool(name="work", bufs=2) as wp, \
         tc.tile_pool(name="psum", bufs=1, space="PSUM") as psp:

        mcat_sb = pp.tile([128, nt_alloc * 32], f32, tag="mcat")
        glo_sb = pp.tile([128, nt_alloc * 16], i16, tag="glo")
        ghi_sb = pp.tile([128, nt_alloc * 16], i16, tag="ghi")
        acc = pp.tile([128, nt_used], f32, tag="acc")

        nc.sync.dma_start(mcat_sb[:], mcat_d[:])
        nc.sync.dma_start(glo_sb[:], glo_d[:])
        nc.sync.dma_start(ghi_sb[:], ghi_d[:])

        # negate the maps_lo half in place: columns t*32 + i*8 + (4..7)
        m0 = mcat_sb[:]
        neg_view = AP(m0.tensor, m0.offset + 4,
                      [m0.ap[0], [32, nt_alloc], [8, D], [1, 4]])
        nc.vector.tensor_scalar(neg_view, neg_view, -1.0, None, Op.mult)

        # double-buffered gather targets: xcat[e, jj, f], jj = 0..3 dst, 4..7 src
        NIDX = 2 * GROUP * 128          # rows per gather
        SCOL = NIDX // 16               # idx columns per gather
        xg_a = gp.tile([128, GROUP * 2 * DF], f32, tag="xg_a")
        xh_a = gp.tile([128, GROUP * 2 * DF], f32, tag="xh_a")
        xg_b = gp.tile([128, GROUP * 2 * DF], f32, tag="xg_b")
        xh_b = gp.tile([128, GROUP * 2 * DF], f32, tag="xh_b")
        # static staging for the (dynamically sliced) int16 index columns
        stl_a = gp.tile([128, SCOL], i16, tag="stl_a")
        sth_a = gp.tile([128, SCOL], i16, tag="sth_a")
        stl_b = gp.tile([128, SCOL], i16, tag="stl_b")
        sth_b = gp.tile([128, SCOL], i16, tag="sth_b")

        def gather(tile0, xg, xh, stl, sth):
            # tile0: first tile index (RuntimeValue or int) of the GROUP.
            # The interleaved index stream makes the gathered rows land as
            # [x_dst | x_src] blocks per tile: row i = (2t+w)*128+p goes to
            # out[p, 2t+w, :].
            col0 = tile0 * 16
            nc.vector.tensor_copy(stl[:], glo_sb[:, ds(col0, SCOL)])
            nc.vector.tensor_copy(sth[:], ghi_sb[:, ds(col0, SCOL)])
            for xv, st, src_d in ((xg, stl, xlo_d), (xh, sth, xhi_d)):
                b = xv[:]
                out3 = AP(b.tensor, b.offset,
                          [b.ap[0], [DF, 2 * GROUP], [1, DF]])
                nc.gpsimd.dma_gather(
                    out_ap=out3, in_ap=src_d[:], idxs_ap=st[:],
                    num_idxs=NIDX, num_idxs_reg=NIDX, elem_size=DF,
                    single_packet=False)
            # merge the two half-table gathers (invalid slots gathered zeros)
            nc.vector.tensor_tensor(xg[:], xg[:], xh[:], Op.add)

        def compute(tile0, xg):
            mc_g = mcat_sb[:, ds(tile0 * 32, GROUP * 32)]
            acc_g = acc[:, ds(tile0, GROUP)]
            for k in range(GROUP):
                prod = wp.tile([128, D * 2 * DF], f32, tag="prod")
                dd = wp.tile([128, DF], f32, tag="dd")
                sq = wp.tile([128, DF], f32, tag="sq")
                xk = xg[:, 2 * DF * k:2 * DF * (k + 1)]
                # in0: xcat[e, (i) jj f] with i broadcast (stride 0)
                in0 = AP(xk.tensor, xk.offset,
                         [xk.ap[0], [0, D], [F, 2 * D], [1, F]])
                mk = mc_g[:, 32 * k:32 * (k + 1)]
                # in1: mcat[e, i jj (f)] with f broadcast (stride 0)
                in1 = AP(mk.tensor, mk.offset,
                         [mk.ap[0], [8, D], [1, 2 * D], [0, F]])
                p0 = prod[:]
                pout = AP(p0.tensor, p0.offset,
                          [p0.ap[0], [2 * DF, D], [F, 2 * D], [1, F]])
                nc.vector.tensor_tensor(pout, in0, in1, Op.mult)
                # reduce over jj (innermost): prod[e, i f jj] -> dd[e, (i f)]
                pin = AP(p0.tensor, p0.offset,
                         [p0.ap[0], [2 * DF, D], [1, F], [F, 2 * D]])
                nc.vector.tensor_reduce(dd[:], pin, axis=mybir.AxisListType.X,
                                        op=Op.add)
                nc.vector.scalar_tensor_tensor(
                    sq[:], dd[:], 0.0, dd[:], Op.bypass, Op.mult,
                    accum_out=acc_g[:, k:k + 1])

        gather(0, xg_a, xh_a, stl_a, sth_a)
        with tc.For_i(0, niters, 1,
                      hint_engines=(mybir.EngineType.DVE,)) as it:
            base = it * PAIR
            gather(base + GROUP, xg_b, xh_b, stl_b, sth_b)
            compute(base, xg_a)
            gather(base + PAIR, xg_a, xh_a, stl_a, sth_a)
            compute(base + GROUP, xg_b)

        colsum = pp.tile([128, 1], f32, tag="colsum")
        ones = pp.tile([128, 1], f32, tag="ones")
        nc.vector.reduce_sum(out=colsum[:], in_=acc[:],
                             axis=mybir.AxisListType.X)
        nc.gpsimd.memset(ones[:], 1.0)
        pt = psp.tile([1, 1], f32, tag="pt")
        nc.tensor.matmul(pt[:], lhsT=colsum[:], rhs=ones[:],
                         start=True, stop=True)
        lsb = pp.tile([1, 1], f32, tag="lsb")
        # *2: each undirected pair contributes both directed edges equally
        nc.vector.tensor_scalar(lsb[:], pt[:], 2.0, None, Op.mult)
        nc.sync.dma_start(loss_d[:], lsb[:])

    nc.compile()
    return nc


_CACHED = {}


def _get_program():
    if "nc" not in _CACHED:
        _inject_axon_hooks()
        _CACHED["nc"] = _build_program()
    return _CACHED["nc"]


def _prep_core_inputs(x_flat, maps3d, src, dst, core):
    """Build the per-core input dict (layout transforms only)."""
    e0 = core * EPC
    e1 = e0 + EPC

    # mcat rows: [e, i, jj]: jj<4 -> maps_hi[e,i,jj], jj>=4 -> maps_lo[e,i,jj-4]
    # (the maps_lo half is negated on device)
    inter = np.zeros((EPC_PAD, D, 8), np.float32)
    inter[:EPC, :, :4] = maps3d[H + e0:H + e1]
    inter[:EPC, :, 4:] = maps3d[e0:e1]
    mcat = np.zeros((128, NT_ALLOC * 32), np.float32)
    mcat[:, :NT_USED * 32] = (
        inter.reshape(NT_USED, 128, 32).transpose(1, 0, 2).reshape(128, -1))

    # linear gather order: i = (2t+w)*128 + p, w=0 dst / w=1 src
    lin = np.full((NT_ALLOC, 2, 128), XSPLIT, np.int32)
    pad = np.zeros(EPC_PAD, np.int32)
    pad[:EPC] = dst[e0:e1]
    lin[:NT_USED, 0, :] = pad.reshape(NT_USED, 128)
    pad = np.zeros(EPC_PAD, np.int32)
    pad[:EPC] = src[e0:e1]
    lin[:NT_USED, 1, :] = pad.reshape(NT_USED, 128)
    lin = lin.reshape(-1)
    lo = np.where(lin < XSPLIT, lin, XSPLIT).astype(np.int16)
    hi = np.where(lin >= XSPLIT, lin - XSPLIT, XSPLIT).astype(np.int16)
    # dma_gather wrapped layout: [16, S] with linear i = s*16 + p,
    # replicated 8x down the partitions
    glo = np.tile(lo.reshape(-1, 16).T, (8, 1))
    ghi = np.tile(hi.reshape(-1, 16).T, (8, 1))

    return {
        "mcat": np.ascontiguousarray(mcat),
        "glo": np.ascontiguousarray(glo),
        "ghi": np.ascontiguousarray(ghi),
    }


def _symmetric_structure(rev_idx):
    r = np.asarray(rev_idx)
    if r.shape != (E,):
        return False
    h = np.arange(H, dtype=r.dtype)
    return bool(np.array_equal(r[:H], h + H) and np.array_equal(r[H:], h))


def _fallback_numpy(x, restriction_maps, edge_index, rev_idx):
    x = np.asarray(x, np.float32)
    maps = np.asarray(restriction_maps, np.float32)
    ei = np.asarray(edge_index)
    rv = np.asarray(rev_idx)
    total = np.float64(0.0)
    chunk = 131072
    ne = ei.shape[1]
    for s in range(0, ne, chunk):
        e = min(s + chunk, ne)
        src = ei[0, s:e]
        tgt = ei[1, s:e]
        fvu = maps[rv[s:e]]
        fuv = maps[s:e]
        t1 = np.einsum("eij,ejf->eif", fvu, x[tgt])
        t2 = np.einsum("eij,ejf->eif", fuv, x[src])
        d = t1 - t2
        total += np.sum((d * d).astype(np.float64))
    return np.float32(total)


def kernel(x, restriction_maps, edge_index, rev_idx):
    x = np.asarray(x)
    restriction_maps = np.asarray(restriction_maps)
    edge_index = np.asarray(edge_index)
    rev_idx = np.asarray(rev_idx)

    if (x.shape != (N, D, F) or restriction_maps.shape != (E, D, D)
            or edge_index.shape != (2, E) or not _symmetric_structure(rev_idx)):
        return _fallback_numpy(x, restriction_maps, edge_index, rev_idx)

    from concourse.bass_utils import run_bass_kernel_spmd

    nc = _get_program()

    x_flat = x.reshape(N, DF).astype(np.float32)
    xlo = np.zeros((XSPLIT + 1, DF), np.float32)
    xlo[:XSPLIT] = x_flat[:XSPLIT]
    xhi = np.zeros((XSPLIT + 1, DF), np.float32)
    xhi[:N - XSPLIT] = x_flat[XSPLIT:]
    maps3d = restriction_maps.astype(np.float32)
    src = edge_index[0].astype(np.int32)
    dst = edge_index[1].astype(np.int32)

    in_maps = []
    for c in range(NCORES):
        m = _prep_core_inputs(x_flat, maps3d, src, dst, c)
        m["xlo"] = xlo
        m["xhi"] = xhi
        in_maps.append(m)
    res = run_bass_kernel_spmd(nc, in_maps, core_ids=list(range(NCORES)))
    total = np.float32(0.0)
    for c in range(NCORES):
        total += res.results[c]["loss"][0, 0]
    return np.float32(total)



# revision 2
# speedup vs baseline: 5.7522x; 5.7522x over previous
"""Trainium2 Bass kernel for sheaf Dirichlet energy (ConsistencyBasedLaplacianBuilder).

loss = sum_e || maps[rev(e)] @ x[tgt(e)] - maps[e] @ x[src(e)] ||_F^2

The reference edge set is symmetric (rev(e) = e+H for e < H = E/2), so
loss = 2 * sum_{e<H} || A_e x_dst - B_e x_src ||^2 with A_e = maps[e+H],
B_e = maps[e].

Strategy (8 cores, dst-range sharding):
  Core c owns the H/8-ish half-edges whose dst lies in [c*6250, (c+1)*6250).
  Per edge the kernel needs two random 256B x-rows; naive indirect DMA costs
  one SWDGE descriptor per row (~8ns descgen on the gpsimd engine) which was
  the previous bottleneck.  This version:

  * dst rows cost NO descriptors: the host bin-packs the core's 6250 dst
    nodes into 49 blocks of <=128 nodes (capacity-aware so each block's
    lo/hi edges fit a fixed tile budget), keeps the core's x-slab resident
    in SBUF (bf16), and gathers the 128 dst rows of each edge-tile with a
    one-hot [128nodes x 128edges] matmul on the (otherwise idle) PE engine.
  * src rows cost ONE descriptor each: edges are partitioned into a "lo"
    section (src < 32767) and "hi" section so a single int16-indexed
    dma_gather from a bf16 x-table (rows padded to 256B) fetches each row
    exactly once; gathers round-robin over 4 SWDGE queues so descriptor
    generation overlaps the DMA drain.
  * compute: per 128-edge tile, one fused [128, 512] bf16 multiply
    prod[e,(j,i,f)] = mcatx[e,(j,i,f)] * xcat[e,(j,f)]   (xcat = [xd|xs])
    with the maps pre-negated/pre-broadcast over f on the host so the DVE
    runs in its 2x packed mode, then a 4-tile-batched strided reduce over j,
    then Square+accumulate on the scalar (Act) engine.
  Per-core scalar losses are summed on the host.
"""

import sys
import types

import numpy as np

sys.path.insert(0, "/opt/trn_rl_repo")

N = 50000
D = 4
F = 16
DF = D * F            # 64 values per node row
E = 1600000
H = E // 2            # 800000 undirected pairs
NCORES = 8

NDST = N // NCORES    # 6250 dst nodes per core
NBLK = 49             # node blocks per core (<=128 nodes each)
TPB_LO = 11           # lo tiles per block (capacity 11*128 lo edges)
TPB_HI = 6            # hi tiles per block
SPLIT = 32767         # src < SPLIT -> lo table, else hi table
NHI = N - SPLIT       # 17233 rows in the hi table

NT_LO = 544           # 49*11 = 539 lo tiles, padded to 34 groups of 16
NT_HI = 304           # 49*6 = 294 hi tiles, padded to 19 groups of 16
NT = NT_LO + NT_HI    # 848 tiles of 128 half-edges
GROUP = 16            # tiles per dma_gather / stream-DMA group
NGRP = NT // GROUP    # 53
NGRP_LO = NT_LO // GROUP  # 34
RB = 4                # tiles per reduce batch
NRB = NT // RB        # 212 accumulator columns


def _inject_axon_hooks():
    """The container's antenv lacks axon_hooks; provide it so NTFF tracing
    (used by test.py, harmless otherwise) can register."""
    if "antenv.axon_hooks" in sys.modules:
        return
    mod = types.ModuleType("antenv.axon_hooks")
    mod._hook = None

    def set_axon_ntff_profile_hook(h):
        mod._hook = h

    def get_axon_ntff_profile_hook():
        return mod._hook

    mod.set_axon_ntff_profile_hook = set_axon_ntff_profile_hook
    mod.get_axon_ntff_profile_hook = get_axon_ntff_profile_hook
    sys.modules["antenv.axon_hooks"] = mod


def _build_program(ncores=NCORES):
    import concourse.bacc as bacc
    import concourse.bass as bass
    import concourse.tile as tile
    from concourse import mybir

    AP = bass.AP
    f32 = mybir.dt.float32
    bf16 = mybir.dt.bfloat16
    i16 = mybir.dt.int16
    Op = mybir.AluOpType

    nc = bacc.Bacc("TRN2", target_bir_lowering=False, debug=False,
                   num_devices=ncores, num_swdge_queues=4)

    slab_d = nc.dram_tensor("slab", [128, NBLK * DF], bf16, kind="ExternalInput")
    idx_d = nc.dram_tensor("idx", [128, NT * 8], i16, kind="ExternalInput")
    oh_d = nc.dram_tensor("oh", [128, NT * 128], bf16, kind="ExternalInput")
    mx_d = nc.dram_tensor("mx", [128, NT * 512], bf16, kind="ExternalInput")
    xlo_d = nc.dram_tensor("xlo", [SPLIT, 128], bf16, kind="ExternalInput")
    xhi_d = nc.dram_tensor("xhi", [NHI, 128], bf16, kind="ExternalInput")
    loss_d = nc.dram_tensor("loss", [1, 1], f32, kind="ExternalOutput")

    # static tile -> node-block schedule (null padding tiles use block 0;
    # their one-hot columns are all-zero so the block choice is irrelevant)
    def blk_of(t):
        if t < NT_LO:
            b = t // TPB_LO
            return b if b < NBLK else 0
        u = t - NT_LO
        b = u // TPB_HI
        return b if b < NBLK else 0

    with tile.TileContext(nc) as tc, \
         tc.tile_pool(name="persist", bufs=1) as pp, \
         tc.tile_pool(name="stream", bufs=2) as sp, \
         tc.tile_pool(name="work", bufs=2) as wp, \
         tc.tile_pool(name="psum", bufs=4, space="PSUM") as psp:

        slab_sb = pp.tile([128, NBLK * DF], bf16, tag="slab")
        idx_sb = pp.tile([128, NT * 8], i16, tag="idx")
        acc = pp.tile([128, NRB], f32, tag="acc")

        nc.sync.dma_start(slab_sb[:], slab_d[:])
        nc.sync.dma_start(idx_sb[:], idx_d[:])

        for g in range(NGRP):
            oh_buf = sp.tile([128, GROUP * 128], bf16, tag="oh")
            mx_buf = sp.tile([128, GROUP * 512], bf16, tag="mx")
            xs_buf = sp.tile([128, GROUP * 128], bf16, tag="xs")

            nc.sync.dma_start(oh_buf[:], oh_d[:, g * GROUP * 128:(g + 1) * GROUP * 128])
            nc.sync.dma_start(mx_buf[:], mx_d[:, g * GROUP * 512:(g + 1) * GROUP * 512])

            xb = xs_buf[:]
            gout = AP(xb.tensor, xb.offset, [xb.ap[0], [128, GROUP], [1, 128]])
            nc.gpsimd.dma_gather(
                out_ap=gout,
                in_ap=(xlo_d[:] if g < NGRP_LO else xhi_d[:]),
                idxs_ap=idx_sb[:, g * 128:(g + 1) * 128],
                num_idxs=GROUP * 128,
                num_idxs_reg=GROUP * 128,
                elem_size=128,
                single_packet=False,
                queue_num=g % 4,
            )

            for rb in range(GROUP // RB):
                pt = psp.tile([128, RB * DF], f32, tag="pt")
                for k in range(RB):
                    t = g * GROUP + rb * RB + k
                    ki = rb * RB + k
                    b = blk_of(t)
                    nc.tensor.matmul(
                        pt[:, k * DF:(k + 1) * DF],
                        lhsT=oh_buf[:, ki * 128:(ki + 1) * 128],
                        rhs=slab_sb[:, b * DF:(b + 1) * DF],
                        start=True, stop=True)

                xcat = wp.tile([128, RB * 128], bf16, tag="xcat")
                xc = xcat[:]
                # dst rows: PSUM f32 -> bf16 into cols [k*128, k*128+64)
                xd_view = AP(xc.tensor, xc.offset, [xc.ap[0], [128, RB], [1, DF]])
                nc.scalar.copy(xd_view, pt[:])
                # src rows: gathered bf16 -> cols [k*128+64, k*128+128)
                xs_view = AP(xc.tensor, xc.offset + DF,
                             [xc.ap[0], [128, RB], [1, DF]])
                xg = xs_buf[:]
                xs_src = AP(xg.tensor, xg.offset + rb * RB * 128,
                            [xg.ap[0], [128, RB], [1, DF]])
                nc.scalar.copy(xs_view, xs_src)

                prod = wp.tile([128, RB * 512], bf16, tag="prod")
                pr = prod[:]
                for k in range(RB):
                    ki = rb * RB + k
                    # in0: xcat[e,(i,j,f)] with i broadcast; j spans [xd|xs]
                    in0 = AP(xc.tensor, xc.offset + k * 128,
                             [xc.ap[0], [0, D], [F, 2 * D], [1, F]])
                    mk = mx_buf[:]
                    # in1: mcatx[e,(i,j,f)] stored (j,i,f)-contiguous
                    in1 = AP(mk.tensor, mk.offset + ki * 512,
                             [mk.ap[0], [F, D], [DF, 2 * D], [1, F]])
                    pout = AP(pr.tensor, pr.offset + k * 512,
                              [pr.ap[0], [F, D], [DF, 2 * D], [1, F]])
                    nc.vector.tensor_tensor(pout, in0, in1, Op.mult)

                dd = wp.tile([128, RB * DF], f32, tag="dd")
                # reduce over j (innermost): prod[e,(k,(i,f),j)] -> dd[e,(k,(i,f))]
                pin = AP(pr.tensor, pr.offset,
                         [pr.ap[0], [512, RB], [1, DF], [DF, 2 * D]])
                nc.vector.tensor_reduce(dd[:], pin, axis=mybir.AxisListType.X,
                                        op=Op.add)
                sq = wp.tile([128, RB * DF], f32, tag="sq")
                nc.scalar.activation(
                    sq[:], dd[:], mybir.ActivationFunctionType.Square,
                    accum_out=acc[:, g * (GROUP // RB) + rb:
                                  g * (GROUP // RB) + rb + 1])

        colsum = pp.tile([128, 1], f32, tag="colsum")
        ones = pp.tile([128, 1], f32, tag="ones")
        nc.vector.reduce_sum(out=colsum[:], in_=acc[:],
                             axis=mybir.AxisListType.X)
        nc.vector.memset(ones[:], 1.0)
        pt11 = psp.tile([1, 1], f32, tag="pt11")
        nc.tensor.matmul(pt11[:], lhsT=colsum[:], rhs=ones[:],
                         start=True, stop=True)
        lsb = pp.tile([1, 1], f32, tag="lsb")
        # *2: each undirected pair contributes both directed edges equally
        nc.vector.tensor_scalar(lsb[:], pt11[:], 2.0, None, Op.mult)
        nc.sync.dma_start(loss_d[:], lsb[:])

    nc.compile()
    return nc


_CACHED = {}


def _get_program():
    if "nc" not in _CACHED:
        _inject_axon_hooks()
        _CACHED["nc"] = _build_program()
    return _CACHED["nc"]


def _pack_nodes(lo_deg, hi_deg):
    """Assign each of the NDST nodes to one of NBLK blocks (<=128 nodes,
    lo-edge sum <= TPB_LO*128, hi-edge sum <= TPB_HI*128).  Returns
    (node_bin, node_pos) or None if infeasible."""
    cap_lo = TPB_LO * 128
    cap_hi = TPB_HI * 128
    tot = lo_deg + hi_deg
    order = np.argsort(-tot, kind="stable")

    # snake (serpentine) assignment over sorted degrees: balances sums
    snake = np.empty(NDST, np.int64)
    pos = np.arange(NDST)
    rnd = pos // NBLK
    col = pos % NBLK
    snake = np.where(rnd % 2 == 0, col, NBLK - 1 - col)
    node_bin = np.empty(NDST, np.int64)
    node_bin[order] = snake
    cnt = np.bincount(node_bin, minlength=NBLK)
    slo = np.bincount(node_bin, weights=lo_deg, minlength=NBLK)
    shi = np.bincount(node_bin, weights=hi_deg, minlength=NBLK)
    if cnt.max() <= 128 and slo.max() <= cap_lo and shi.max() <= cap_hi:
        pass
    else:
        # greedy LPT fallback
        rem_n = np.full(NBLK, 128, np.int64)
        rem_lo = np.full(NBLK, cap_lo, np.int64)
        rem_hi = np.full(NBLK, cap_hi, np.int64)
        node_bin = np.empty(NDST, np.int64)
        for v in order:
            ok = (rem_n > 0) & (rem_lo >= lo_deg[v]) & (rem_hi >= hi_deg[v])
            if not ok.any():
                return None
            k = int(np.argmax(np.where(ok, rem_lo + rem_hi, -1)))
            node_bin[v] = k
            rem_n[k] -= 1
            rem_lo[k] -= lo_deg[v]
            rem_hi[k] -= hi_deg[v]

    # position within block (order of assignment is irrelevant)
    order2 = np.argsort(node_bin, kind="stable")
    within = np.arange(NDST) - np.concatenate(
        [[0], np.cumsum(np.bincount(node_bin, minlength=NBLK))])[:-1][
        node_bin[order2]]
    node_pos = np.empty(NDST, np.int64)
    node_pos[order2] = within
    return node_bin, node_pos


def _slot_edges(blk, tpb, tile_base):
    """Assign edges (given their block ids) to tile slots.
    Returns global slot index per edge, or None on overflow."""
    order = np.argsort(blk, kind="stable")
    counts = np.bincount(blk, minlength=NBLK)
    if (counts > tpb * 128).any():
        return None
    starts = np.concatenate([[0], np.cumsum(counts)])[:-1]
    within = np.arange(len(blk)) - starts[blk[order]]
    tiles = tile_base + blk[order] * tpb + within // 128
    slots = tiles * 128 + within % 128
    out = np.empty(len(blk), np.int64)
    out[order] = slots
    return out


def _prep_core_inputs(x_bf, maps3d, src, dst, core):
    """Build the per-core input dict (host-side layout transforms only).
    Returns None if the static tile schedule cannot hold this core's edges."""
    import ml_dtypes
    bf16 = ml_dtypes.bfloat16

    d = dst[:H]
    s = src[:H]
    mask = (d // NDST) == core
    dc = d[mask].astype(np.int64)
    sc = s[mask].astype(np.int64)
    eidx = np.flatnonzero(mask)
    dloc = dc - core * NDST

    is_lo = sc < SPLIT
    lo_deg = np.bincount(dloc[is_lo], minlength=NDST)
    hi_deg = np.bincount(dloc[~is_lo], minlength=NDST)
    packed = _pack_nodes(lo_deg, hi_deg)
    if packed is None:
        return None
    node_bin, node_pos = packed

    eb = node_bin[dloc]
    slots = np.full(len(dc), -1, np.int64)
    r = _slot_edges(eb[is_lo], TPB_LO, 0)
    if r is None:
        return None
    slots[is_lo] = r
    r = _slot_edges(eb[~is_lo], TPB_HI, NT_LO)
    if r is None:
        return None
    slots[~is_lo] = r

    # src index stream (int16, dma_gather wrapped layout)
    lin = np.zeros(NT * 128, np.int16)
    lin[slots[is_lo]] = sc[is_lo].astype(np.int16)
    lin[slots[~is_lo]] = (sc[~is_lo] - SPLIT).astype(np.int16)
    idx = np.tile(lin.reshape(-1, 16).T, (8, 1))

    # one-hot dst selectors
    oh = np.zeros((128, NT * 128), bf16)
    oh[node_pos[dloc], slots] = 1

    # maps, (j,i,f) layout, B pre-negated, broadcast over f, bf16
    A = maps3d[H + eidx]          # [m, 4, 4]
    B = maps3d[eidx]
    m8 = np.zeros((NT * 128, 2 * D, D), np.float32)
    m8[slots, :D, :] = A.transpose(0, 2, 1)
    m8[slots, D:, :] = -B.transpose(0, 2, 1)
    mxb = np.broadcast_to(m8.astype(bf16)[..., None],
                          (NT * 128, 2 * D, D, F))
    mx = np.ascontiguousarray(
        mxb.reshape(NT, 128, 512).transpose(1, 0, 2)).reshape(128, NT * 512)

    # SBUF-resident x slab for this core's dst range
    slab = np.zeros((128, NBLK * DF), bf16)
    cols = node_bin[:, None] * DF + np.arange(DF)[None, :]
    slab[node_pos[:, None], cols] = x_bf[core * NDST:(core + 1) * NDST]

    return {
        "slab": slab,
        "idx": np.ascontiguousarray(idx),
        "oh": oh,
        "mx": mx,
    }


def _prepare_all(x, restriction_maps, edge_index):
    """Host prep for all cores; returns in_maps list or None if infeasible."""
    import ml_dtypes
    bf16 = ml_dtypes.bfloat16

    x_flat = np.ascontiguousarray(x.reshape(N, DF)).astype(np.float32)
    x_bf = x_flat.astype(bf16)
    xlo = np.zeros((SPLIT, 128), bf16)
    xlo[:, :DF] = x_bf[:SPLIT]
    xhi = np.zeros((NHI, 128), bf16)
    xhi[:, :DF] = x_bf[SPLIT:]
    maps3d = restriction_maps.astype(np.float32)
    src = edge_index[0].astype(np.int64)
    dst = edge_index[1].astype(np.int64)

    in_maps = []
    for c in range(NCORES):
        m = _prep_core_inputs(x_bf, maps3d, src, dst, c)
        if m is None:
            return None
        m["xlo"] = xlo
        m["xhi"] = xhi
        in_maps.append(m)
    return in_maps


def _symmetric_structure(rev_idx):
    r = np.asarray(rev_idx)
    if r.shape != (E,):
        return False
    h = np.arange(H, dtype=r.dtype)
    return bool(np.array_equal(r[:H], h + H) and np.array_equal(r[H:], h))


def _fallback_numpy(x, restriction_maps, edge_index, rev_idx):
    x = np.asarray(x, np.float32)
    maps = np.asarray(restriction_maps, np.float32)
    ei = np.asarray(edge_index)
    rv = np.asarray(rev_idx)
    total = np.float64(0.0)
    chunk = 131072
    ne = ei.shape[1]
    for st in range(0, ne, chunk):
        en = min(st + chunk, ne)
        srcc = ei[0, st:en]
        tgt = ei[1, st:en]
        fvu = maps[rv[st:en]]
        fuv = maps[st:en]
        t1 = np.einsum("eij,ejf->eif", fvu, x[tgt])
        t2 = np.einsum("eij,ejf->eif", fuv, x[srcc])
        dd = t1 - t2
        total += np.sum((dd * dd).astype(np.float64))
    return np.float32(total)


def kernel(x, restriction_maps, edge_index, rev_idx):
    x = np.asarray(x)
    restriction_maps = np.asarray(restriction_maps)
    edge_index = np.asarray(edge_index)
    rev_idx = np.asarray(rev_idx)

    if (x.shape != (N, D, F) or restriction_maps.shape != (E, D, D)
            or edge_index.shape != (2, E) or not _symmetric_structure(rev_idx)):
        return _fallback_numpy(x, restriction_maps, edge_index, rev_idx)

    in_maps = _prepare_all(x, restriction_maps, edge_index)
    if in_maps is None:
        return _fallback_numpy(x, restriction_maps, edge_index, rev_idx)

    from concourse.bass_utils import run_bass_kernel_spmd

    nc = _get_program()
    res = run_bass_kernel_spmd(nc, in_maps, core_ids=list(range(NCORES)))
    total = np.float32(0.0)
    for c in range(NCORES):
        total += res.results[c]["loss"][0, 0]
    return np.float32(total)
